# revision 46
# baseline (speedup 1.0000x reference)
"""Cross-attention (S2Audio) Trainium2 Bass kernel.

Sharding: data-parallel over the clip batch B=8 -> one batch element per
NeuronCore.  Per core, for its batch element b:

  q = (audio_patch + pos_a) @ q_w.T + q_b          (1568, 768)
  k,v = (s_x_patch + pos_s) @ kv_w.T + kv_b        (1568, 768) each
  out = softmax(q k^T / sqrt(64)) v  per 12 heads  -> proj -> (1568, 768)

Engine strategy (per core):
  * All projections + scores matmuls in bf16 (1 cycle/row on PE), fp32 PSUM.
  * PV matmul in fp8 with DoubleRow perf mode (0.5 cycles/row): V in e4m3,
    interleaved [128, 2, H, 128] per 256-key pair (v dims 0-63, ones at
    col 64 -> softmax denominator in pv row 64, zero padding above — DR
    weight tiles must be exactly 32/64/128 columns).  exp tiles are e5m2
    in the DoubleRow rhs layout [128, 2, nq]: e5m2's 2^31 dynamic range
    covers exp of the full score range (+-9.5) with no max-subtraction
    and no overflow (e4m3 exp overflowed to NaN bits / flushed whole
    rows to zero).
  * q_feat is pre-scaled by A8 = SCALE*4*log2(e) so the scores PSUM holds
    s*A8.  exp splits across the two PSUM-capable engines (GpSimd cannot
    read PSUM):
      - ACT:  exact exp via activation(Exp, scale=ln2/4) -> e5m2 out
      - DVE:  Schraudolph bit trick: min(round(psum + B8), 123) as
              saturating-uint8 through a bitcast view = e5m2 bits of exp
  * Softmax normalization: DVE reciprocal of the denominator row, GpSimd
    partition_broadcast (SBUF only), DVE scalar_tensor_tensor multiply.
  * One flat pipeline, no phase drains: scores/PV software-pipelined by
    one head; Q-proj of the next block and O-proj of the previous block
    are interleaved into the head loop as PE filler so the PE never
    outruns the exp engines (PSUM pair rotation stays unblocked) and the
    PE p-state stays at full clock.  DMAs are consolidated (HWDGE costs
    ~625ns per dma_start regardless of size).
  * PSUM budget: score-pair tiles [128, 2, 512] x3 (6 banks) + pv x2 = 8.
  * TimelineSim: 292.3us/core (baseline 458.1us); measured rel err 9.9e-3.
"""

import numpy as np
from contextlib import ExitStack

B, T, NPATCH, APATCH, D, H = 8, 8, 196, 196, 768, 12
HD = D // H                      # 64
SCALE = float(HD) ** -0.5        # 0.125
NT = NPATCH * T                  # 1568 tokens (both q and kv side)
P = 128
DC = D // P                      # 6 feature chunks
N_CORES = 8

NPAIR = 6                        # 6 x 256-key DoubleRow pairs
REM0 = NPAIR * 256               # 1536
REMW = NT - REM0                 # 32 remainder keys
NQB = 512
NQ_BLOCKS = [(s, min(NQB, NT - s)) for s in range(0, NT, NQB)]
TOK_CHUNKS = [(i * P, min(P, NT - i * P)) for i in range((NT + P - 1) // P)]

A8 = SCALE * 4.0 * float(np.log2(np.e))   # q prescale so psum = s_raw*A8
LN2_8 = float(np.log(2.0)) / 4.0          # ACT exp scale on prescaled psum
B8 = 60.0 - 0.5                           # Schraudolph e5m2 offset (tuned)
B8CLIP = 123.0                            # e5m2 bits >= 124 are inf/nan

_CACHE: dict = {}


def _build_nc(qb_nz: bool, kb_nz: bool, vb_nz: bool, pb_nz: bool):
    import concourse.mybir as mybir
    from concourse import bacc
    from concourse.tile import TileContext

    f32 = mybir.dt.float32
    bf16 = mybir.dt.bfloat16
    e4 = mybir.dt.float8e4
    e5 = mybir.dt.float8e5
    u8 = mybir.dt.uint8
    AF = mybir.ActivationFunctionType
    Alu = mybir.AluOpType
    DR = mybir.MatmulPerfMode.DoubleRow

    nc = bacc.Bacc("TRN2", target_bir_lowering=False, debug=False,
                   num_devices=N_CORES)

    xsT = nc.dram_tensor("xsT", [D, NT], bf16, kind="ExternalInput")
    xaT = nc.dram_tensor("xaT", [D, NT], bf16, kind="ExternalInput")
    qwT = nc.dram_tensor("qwT", [D, D], bf16, kind="ExternalInput")
    kvwT = nc.dram_tensor("kvwT", [D, 2 * D], bf16, kind="ExternalInput")
    projT = nc.dram_tensor("projT", [D, D], bf16, kind="ExternalInput")
    qb = nc.dram_tensor("qb", [P, DC], f32, kind="ExternalInput") if qb_nz else None
    kb = nc.dram_tensor("kb", [P, DC], f32, kind="ExternalInput") if kb_nz else None
    vb = nc.dram_tensor("vb", [1, D], bf16, kind="ExternalInput") if vb_nz else None
    pb = nc.dram_tensor("pb", [1, D], bf16, kind="ExternalInput") if pb_nz else None
    out = nc.dram_tensor("out", [NT, D], f32, kind="ExternalOutput")

    kvwR = kvwT.rearrange("(c p) d -> p c d", p=P)

    with TileContext(nc) as tc, ExitStack() as ctx:
        consts = ctx.enter_context(tc.tile_pool(name="consts", bufs=1))
        persist = ctx.enter_context(tc.tile_pool(name="persist", bufs=1))
        wtp = ctx.enter_context(tc.tile_pool(name="wtp", bufs=1))
        xfp = ctx.enter_context(tc.tile_pool(name="xfp", bufs=1))
        xfb = ctx.enter_context(tc.tile_pool(name="xfb", bufs=2))
        qfb = ctx.enter_context(tc.tile_pool(name="qfb", bufs=2))
        expp = ctx.enter_context(tc.tile_pool(name="expp", bufs=2))
        ofp = ctx.enter_context(tc.tile_pool(name="ofp", bufs=2))
        otp = ctx.enter_context(tc.tile_pool(name="otp", bufs=2))
        nrm = ctx.enter_context(tc.tile_pool(name="nrm", bufs=2))
        ps2 = ctx.enter_context(tc.tile_pool(name="ps2", bufs=3, space="PSUM"))
        pvps = ctx.enter_context(tc.tile_pool(name="pvps", bufs=2, space="PSUM"))

        ones_bf = consts.tile([1, P], bf16, tag="ones_bf")
        nc.gpsimd.memset(ones_bf[:], 1.0)
        qb_sb = kb_sb = vb_sb = pb_sb = None
        if qb_nz:
            qb_sb = consts.tile([P, DC], f32, tag="qb")
            nc.sync.dma_start(qb_sb[:], qb[:])
        if kb_nz:
            kb_sb = consts.tile([P, DC], f32, tag="kb")
            nc.sync.dma_start(kb_sb[:], kb[:])
        if vb_nz:
            vb_sb = consts.tile([1, D], bf16, tag="vb")
            nc.sync.dma_start(vb_sb[:], vb[:])
        if pb_nz:
            pb_sb = consts.tile([1, D], bf16, tag="pb")
            nc.sync.dma_start(pb_sb[:], pb[:])

        # persistent tensors
        k_feat = [persist.tile([P, NT], bf16, tag=f"k_feat{c}", name=f"k_feat{c}")
                  for c in range(DC)]
        v_pair = [persist.tile([P, 2, H, P], e4, tag=f"v{i}", name=f"v{i}")
                  for i in range(NPAIR)]
        v_rem = persist.tile([REMW, H, P], e4, tag="vrem", name="v_rem")
        for i in range(NPAIR):
            nc.gpsimd.memset(v_pair[i][:, :, :, HD:], 0.0)
            nc.gpsimd.memset(v_pair[i][:, :, :, HD:HD + 1], 1.0)
        nc.gpsimd.memset(v_rem[:, :, HD:], 0.0)
        nc.gpsimd.memset(v_rem[:, :, HD:HD + 1], 1.0)

        kvw_sb = wtp.tile([P, DC, 2 * D], bf16, tag="kvw", name="kvw")
        qw_sb = wtp.tile([P, DC, D], bf16, tag="qw", name="qw")
        pw_sb = wtp.tile([P, DC, D], bf16, tag="pw", name="pw")
        xs_feat = xfp.tile([P, DC, NT], bf16, tag="xsf", name="xsf")

        # ---- DMA emission order = execution order on the queue ----
        # xs block 0 slices + kvw K-half m-chunks first so K-proj starts
        # ~3us in; everything else follows in need-order.
        xsR = xsT.rearrange("(c p) n -> p c n", p=P)
        (n0_0, nw_0) = NQ_BLOCKS[0]
        nc.sync.dma_start(kvw_sb[:, :, :D // 2], kvwR[:, :, :D // 2])
        nc.sync.dma_start(xs_feat[:, :, n0_0:n0_0 + nw_0],
                          xsR[:, :, n0_0:n0_0 + nw_0])
        nc.sync.dma_start(kvw_sb[:, :, D // 2:D], kvwR[:, :, D // 2:D])
        nc.sync.dma_start(xs_feat[:, :, nw_0:], xsR[:, :, nw_0:])
        nc.sync.dma_start(qw_sb[:], qwT.rearrange("(c p) d -> p c d", p=P))
        nc.sync.dma_start(kvw_sb[:, :, D:], kvwR[:, :, D:])
        nc.sync.dma_start(pw_sb[:], projT.rearrange("(c p) d -> p c d", p=P))

        xa_feat = {}   # b -> list of tiles
        q_feat = {}    # b -> list of tiles

        xaR = xaT.rearrange("(c p) n -> p c n", p=P)

        def emit_xa_dma(b):
            n0, nw = NQ_BLOCKS[b]
            t = xfb.tile([P, DC, NQB], bf16, tag="xaf", name="xaf")
            nc.sync.dma_start(t[:, :, :nw], xaR[:, :, n0:n0 + nw])
            xa_feat[b] = t

        # ---------------- K projection (feature-major bf16) ----------------
        for (n0, nw) in NQ_BLOCKS:
            for mp in range(DC // 2):
                ps = ps2.tile([P, 2, NQB], f32, tag="pair", name="kproj")
                for j in range(2):
                    m = 2 * mp + j
                    for c in range(DC):
                        nc.tensor.matmul(ps[:, j, :nw],
                                         kvw_sb[:, c, m * P:(m + 1) * P],
                                         xs_feat[:, c, n0:n0 + nw],
                                         start=(c == 0), stop=(c == DC - 1))
                for j in range(2):
                    m = 2 * mp + j
                    dst = k_feat[m][:, n0:n0 + nw]
                    if kb_nz:
                        nc.vector.tensor_scalar(
                            dst, ps[:, j, :nw], kb_sb[:, m:m + 1], None, Alu.add)
                    else:
                        nc.vector.tensor_copy(dst, ps[:, j, :nw])

        # ---------------- Q projection for one block ----------------
        def emit_qproj(b, mp):
            n0, nw = NQ_BLOCKS[b]
            if mp == 0:
                q_feat[b] = [qfb.tile([P, NQB], bf16, tag=f"qf{c}", name=f"qf{c}")
                             for c in range(DC)]
            ps = ps2.tile([P, 2, NQB], f32, tag="pair", name="qproj")
            for j in range(2):
                m = 2 * mp + j
                for c in range(DC):
                    nc.tensor.matmul(ps[:, j, :nw],
                                     qw_sb[:, c, m * P:(m + 1) * P],
                                     xa_feat[b][:, c, :nw],
                                     start=(c == 0), stop=(c == DC - 1))
            for j in range(2):
                m = 2 * mp + j
                if qb_nz:
                    nc.scalar.activation(q_feat[b][m][:, :nw], ps[:, j, :nw],
                                         AF.Identity, bias=qb_sb[:, m:m + 1],
                                         scale=A8)
                else:
                    nc.scalar.activation(q_feat[b][m][:, :nw], ps[:, j, :nw],
                                         AF.Copy, scale=A8)

        # ---------------- V projection -> fp8 interleaved ----------------
        for ti, (t0, tw) in enumerate(TOK_CHUNKS):
            ps = ps2.tile([P, 2, NQB], f32, tag="pair", name="vproj")
            for half in range(2):
                for c in range(DC):
                    nc.tensor.matmul(
                        ps[:tw, half, :384],
                        xs_feat[:, c, t0:t0 + tw],
                        kvw_sb[:, c, D + half * 384:D + (half + 1) * 384],
                        start=(c == 0), stop=(c == DC - 1 and not vb_nz))
                if vb_nz:
                    nc.tensor.matmul(
                        ps[:tw, half, :384], ones_bf[:, :tw],
                        vb_sb[:, half * 384:(half + 1) * 384],
                        start=False, stop=True)
                if ti < 2 * NPAIR:
                    dst = v_pair[ti // 2][:tw, ti % 2,
                                          half * 6:(half + 1) * 6, :HD]
                else:
                    dst = v_rem[:tw, half * 6:(half + 1) * 6, :HD]
                nc.scalar.activation(
                    dst, ps[:tw, half, :384].rearrange("p (h d) -> p h d", d=HD),
                    AF.Copy)

        out_feat = {}  # b -> list of tiles
        ostate = {}    # (b, chunk) -> (ps, ot)

        def emit_ohalf(b, chunk, half):
            n0, nw = NQ_BLOCKS[b]
            c0 = chunk * P
            cw = min(P, nw - c0)
            if cw <= 0:
                return
            if half == 0:
                ps = ps2.tile([P, 2, NQB], f32, tag="pair", name="oproj")
                ot = otp.tile([P, D], f32, tag="ot", name="ot")
                ostate[(b, chunk)] = (ps, ot)
            ps, ot = ostate[(b, chunk)]
            for c in range(DC):
                nc.tensor.matmul(
                    ps[:cw, half, :384],
                    out_feat[b][c][:, c0:c0 + cw],
                    pw_sb[:, c, half * 384:(half + 1) * 384],
                    start=(c == 0), stop=(c == DC - 1 and not pb_nz))
            if pb_nz:
                nc.tensor.matmul(
                    ps[:cw, half, :384], ones_bf[:, :cw],
                    pb_sb[:, half * 384:(half + 1) * 384],
                    start=False, stop=True)
            if half == 0:
                nc.scalar.activation(ot[:cw, :384], ps[:cw, 0, :384], AF.Copy)
            else:
                nc.vector.tensor_copy(ot[:cw, 384:], ps[:cw, 1, :384])
                nc.sync.dma_start(out[n0 + c0:n0 + c0 + cw, :], ot[:cw, :])
                del ostate[(b, chunk)]

        # ---------------- flat block pipeline ----------------
        emit_xa_dma(0)
        for mp in range(DC // 2):
            emit_qproj(0, mp)

        NB = len(NQ_BLOCKS)
        for b in range(NB):
            n0, nw = NQ_BLOCKS[b]
            if b + 1 < NB:
                emit_xa_dma(b + 1)
            out_feat[b] = [ofp.tile([P, NQB], bf16, tag=f"of{c}", name=f"of{c}")
                           for c in range(DC)]

            # PE filler slots: O-proj halves of block b-1 + Q-proj m-pairs
            # of block b+1, spread across the head pipeline.
            slots = []
            if b > 0:
                pn0, pnw = NQ_BLOCKS[b - 1]
                nchunks = (pnw + P - 1) // P
                for chunk in range(nchunks):
                    slots.append(("o", b - 1, chunk, 0))
                    slots.append(("o", b - 1, chunk, 1))
            if b + 1 < NB:
                for mp in range(DC // 2):
                    slots.append(("q", b + 1, mp, None))


            exps = {}
            pvs = {}
            bcs = {}
            si = 0
            for it in range(H + 2):
                if it >= 2:
                    h = it - 2
                    hc, hp = h // 2, (h % 2) * HD
                    nc.vector.scalar_tensor_tensor(
                        out_feat[b][hc][hp:hp + HD, :nw],
                        pvs.pop(h)[:HD, :nw], 1.0, bcs.pop(h)[:, :nw],
                        Alu.mult, Alu.mult)

                def emit_score_pair(h, i):
                    hc, hp = h // 2, (h % 2) * HD
                    ps = ps2.tile([P, 2, NQB], f32, tag="pair", name="score")
                    for j in range(2):
                        t0 = 256 * i + 128 * j
                        nc.tensor.matmul(
                            ps[:, j, :nw],
                            k_feat[hc][hp:hp + HD, t0:t0 + P],
                            q_feat[b][hc][hp:hp + HD, :nw],
                            start=True, stop=True)
                    et = expp.tile([P, 2, NQB], e5, tag=f"e{i}", name="exp")
                    if i in (0, 3):
                        nc.vector.tensor_scalar(
                            et.bitcast(u8)[:, :, :nw], ps[:, :, :nw],
                            B8, B8CLIP, Alu.add, Alu.min)
                    else:
                        nc.scalar.activation(et[:, :, :nw], ps[:, :, :nw],
                                             AF.Exp, scale=LN2_8)
                    return et

                def emit_score_rem(h):
                    hc, hp = h // 2, (h % 2) * HD
                    psr = pvps.tile([P, NQB], f32, tag="pv", name="screm")
                    nc.tensor.matmul(psr[:REMW, :nw],
                                     k_feat[hc][hp:hp + HD, REM0:NT],
                                     q_feat[b][hc][hp:hp + HD, :nw],
                                     start=True, stop=True)
                    etr = expp.tile([REMW, NQB], e5, tag="er", name="expr")
                    if h % 2 == 0:
                        nc.vector.tensor_scalar(
                            etr.bitcast(u8)[:, :nw], psr[:REMW, :nw],
                            B8, B8CLIP, Alu.add, Alu.min)
                    else:
                        nc.scalar.activation(etr[:, :nw], psr[:REMW, :nw],
                                             AF.Exp, scale=LN2_8)
                    return etr

                # scores for head `it`
                if it < H:
                    h = it
                    exp_t = [emit_score_pair(h, i) for i in range(NPAIR)]
                    exps[h] = (exp_t, emit_score_rem(h))

                # one PE filler slot per iteration
                if si < len(slots):
                    kind, bb, xx, yy = slots[si]
                    si += 1
                    if kind == "o":
                        emit_ohalf(bb, xx, yy)
                    else:
                        emit_qproj(bb, xx)

                # PV + normalization chain for head it-1
                if 1 <= it <= H:
                    hh = it - 1
                    pexp_t, petr = exps.pop(hh)
                    pv = pvps.tile([P, NQB], f32, tag="pv", name="pv")
                    for i in range(NPAIR):
                        nc.tensor.matmul(pv[:, :nw],
                                         v_pair[i][:, :, hh, :],
                                         pexp_t[i][:, :, :nw],
                                         start=(i == 0), stop=False,
                                         perf_mode=DR)
                    nc.tensor.matmul(pv[:, :nw], v_rem[:, hh, :],
                                     petr[:, :nw], start=False, stop=True)
                    pvs[hh] = pv
                    rec = nrm.tile([1, NQB], f32, tag="rec", name="rec")
                    nc.vector.reciprocal(rec[:, :nw], pv[HD:HD + 1, :nw])
                    bc = nrm.tile([HD, NQB], f32, tag="bc", name="bc", bufs=3)
                    nc.gpsimd.partition_broadcast(bc[:, :nw], rec[:, :nw])
                    bcs[hh] = bc

            while si < len(slots):
                kind, bb, xx, yy = slots[si]
                si += 1
                if kind == "o":
                    emit_ohalf(bb, xx, yy)
                else:
                    emit_qproj(bb, xx)

        # epilogue: O-projection of the final block
        bl = NB - 1
        n0, nw = NQ_BLOCKS[bl]
        for chunk in range((nw + P - 1) // P):
            emit_ohalf(bl, chunk, 0)
            emit_ohalf(bl, chunk, 1)

    nc.finalize()
    return nc


def kernel(**inputs) -> np.ndarray:
    import ml_dtypes
    bf = ml_dtypes.bfloat16

    s_x = np.asarray(inputs["s_x"], np.float32)
    audio = np.asarray(inputs["audio"], np.float32)
    q_w = np.asarray(inputs["q_w"], np.float32)
    q_b = np.asarray(inputs["q_b"], np.float32)
    kv_w = np.asarray(inputs["kv_w"], np.float32)
    kv_b = np.asarray(inputs["kv_b"], np.float32)
    proj_w = np.asarray(inputs["proj_w"], np.float32)
    proj_b = np.asarray(inputs["proj_b"], np.float32)

    # host prep: layout + O(N*D) positional add + bf16 casts only
    pos_s = (np.asarray(inputs["clip_space_pos"], np.float32)[:, None, :]
             + np.asarray(inputs["clip_temporal_pos"], np.float32)[None, :, :]
             ).reshape(NT, D)
    pos_a = (np.asarray(inputs["audio_space_pos"], np.float32)[:, None, :]
             + np.asarray(inputs["audio_temporal_pos"], np.float32)[None, :, :]
             ).reshape(NT, D)
    qwT = np.ascontiguousarray(q_w.T).astype(bf)
    kvwT = np.ascontiguousarray(kv_w.T).astype(bf)
    projT = np.ascontiguousarray(proj_w.T).astype(bf)
    qb_nz = bool(np.any(q_b))
    kb_nz = bool(np.any(kv_b[:D]))
    vb_nz = bool(np.any(kv_b[D:]))
    pb_nz = bool(np.any(proj_b))

    key = (qb_nz, kb_nz, vb_nz, pb_nz)
    if key not in _CACHE:
        _CACHE[key] = _build_nc(*key)
    nc = _CACHE[key]

    shared = {"qwT": qwT, "kvwT": kvwT, "projT": projT}
    if qb_nz:
        shared["qb"] = np.ascontiguousarray(
            (q_b * A8).reshape(DC, P).T.astype(np.float32))
    if kb_nz:
        shared["kb"] = np.ascontiguousarray(kv_b[:D].reshape(DC, P).T)
    if vb_nz:
        shared["vb"] = np.ascontiguousarray(kv_b[D:].reshape(1, D)).astype(bf)
    if pb_nz:
        shared["pb"] = np.ascontiguousarray(proj_b.reshape(1, D)).astype(bf)

    in_maps = []
    for b in range(N_CORES):
        m = dict(shared)
        m["xsT"] = np.ascontiguousarray(
            (s_x[1:, b * T:(b + 1) * T, :].reshape(NT, D) + pos_s).T).astype(bf)
        m["xaT"] = np.ascontiguousarray(
            (audio[2:, b * T:(b + 1) * T, :].reshape(NT, D) + pos_a).T).astype(bf)
        in_maps.append(m)

    from concourse.bass_utils import run_bass_kernel_spmd
    res = run_bass_kernel_spmd(nc, in_maps, core_ids=list(range(N_CORES)))
    global LAST_RESULTS
    LAST_RESULTS = res

    out_full = np.empty((2 + APATCH, B * T, D), np.float32)
    out_full[:2] = audio[:2]
    for b in range(N_CORES):
        out_full[2:, b * T:(b + 1) * T, :] = \
            res.results[b]["out"].reshape(APATCH, T, D)
    return out_full


# revision 48
# speedup vs baseline: 1.0066x; 1.0066x over previous
"""Cross-attention (S2Audio) Trainium2 Bass kernel.

Sharding: data-parallel over the clip batch B=8 -> one batch element per
NeuronCore.  Per core, for its batch element b:

  q = (audio_patch + pos_a) @ q_w.T + q_b          (1568, 768)
  k,v = (s_x_patch + pos_s) @ kv_w.T + kv_b        (1568, 768) each
  out = softmax(q k^T / sqrt(64)) v  per 12 heads  -> proj -> (1568, 768)

Engine strategy (per core):
  * All projections + scores matmuls in bf16 (1 cycle/row on PE), fp32 PSUM.
  * PV matmul in fp8 with DoubleRow perf mode (0.5 cycles/row): V in e4m3,
    interleaved [128, 2, H, 128] per 256-key pair (v dims 0-63, ones at
    col 64 -> softmax denominator in pv row 64, zero padding above — DR
    weight tiles must be exactly 32/64/128 columns).  exp tiles are e5m2
    in the DoubleRow rhs layout [128, 2, nq]: e5m2's 2^31 dynamic range
    covers exp of the full score range (+-9.5) with no max-subtraction
    and no overflow (e4m3 exp overflowed to NaN bits / flushed whole
    rows to zero).
  * q_feat is pre-scaled by A8 = SCALE*4*log2(e) so the scores PSUM holds
    s*A8.  exp splits across the two PSUM-capable engines (GpSimd cannot
    read PSUM):
      - ACT:  exact exp via activation(Exp, scale=ln2/4) -> e5m2 out
      - DVE:  Schraudolph bit trick: min(round(psum + B8), 123) as
              saturating-uint8 through a bitcast view = e5m2 bits of exp
  * Softmax normalization: DVE reciprocal of the denominator row, GpSimd
    partition_broadcast (SBUF only), DVE scalar_tensor_tensor multiply.
  * One flat pipeline, no phase drains: scores/PV software-pipelined by
    one head; Q-proj of the next block and O-proj of the previous block
    are interleaved into the head loop as PE filler so the PE never
    outruns the exp engines (PSUM pair rotation stays unblocked) and the
    PE p-state stays at full clock.  DMAs are consolidated (HWDGE costs
    ~625ns per dma_start regardless of size).
  * PSUM budget: score-pair tiles [128, 2, 512] x3 (6 banks) + pv x2 = 8.
  * TimelineSim: 292.3us/core (baseline 458.1us); measured rel err 9.9e-3.
"""

import numpy as np
from contextlib import ExitStack

B, T, NPATCH, APATCH, D, H = 8, 8, 196, 196, 768, 12
HD = D // H                      # 64
SCALE = float(HD) ** -0.5        # 0.125
NT = NPATCH * T                  # 1568 tokens (both q and kv side)
P = 128
DC = D // P                      # 6 feature chunks
N_CORES = 8

NPAIR = 6                        # 6 x 256-key DoubleRow pairs
REM0 = NPAIR * 256               # 1536
REMW = NT - REM0                 # 32 remainder keys
NQB = 512
NQ_BLOCKS = [(s, min(NQB, NT - s)) for s in range(0, NT, NQB)]
TOK_CHUNKS = [(i * P, min(P, NT - i * P)) for i in range((NT + P - 1) // P)]

A8 = SCALE * 4.0 * float(np.log2(np.e))   # q prescale so psum = s_raw*A8
LN2_8 = float(np.log(2.0)) / 4.0          # ACT exp scale on prescaled psum
B8 = 60.0 - 0.5                           # Schraudolph e5m2 offset (tuned)
B8CLIP = 123.0                            # e5m2 bits >= 124 are inf/nan

_CACHE: dict = {}


def _build_nc(qb_nz: bool, kb_nz: bool, vb_nz: bool, pb_nz: bool):
    import concourse.mybir as mybir
    from concourse import bacc
    from concourse.tile import TileContext

    f32 = mybir.dt.float32
    bf16 = mybir.dt.bfloat16
    e4 = mybir.dt.float8e4
    e5 = mybir.dt.float8e5
    u8 = mybir.dt.uint8
    AF = mybir.ActivationFunctionType
    Alu = mybir.AluOpType
    DR = mybir.MatmulPerfMode.DoubleRow

    nc = bacc.Bacc("TRN2", target_bir_lowering=False, debug=False,
                   num_devices=N_CORES)

    xsT = nc.dram_tensor("xsT", [D, NT], bf16, kind="ExternalInput")
    xaT = nc.dram_tensor("xaT", [D, NT], bf16, kind="ExternalInput")
    qwT = nc.dram_tensor("qwT", [D, D], bf16, kind="ExternalInput")
    kvwT = nc.dram_tensor("kvwT", [D, 2 * D], bf16, kind="ExternalInput")
    projT = nc.dram_tensor("projT", [D, D], bf16, kind="ExternalInput")
    qb = nc.dram_tensor("qb", [P, DC], f32, kind="ExternalInput") if qb_nz else None
    kb = nc.dram_tensor("kb", [P, DC], f32, kind="ExternalInput") if kb_nz else None
    vb = nc.dram_tensor("vb", [1, D], bf16, kind="ExternalInput") if vb_nz else None
    pb = nc.dram_tensor("pb", [1, D], bf16, kind="ExternalInput") if pb_nz else None
    out = nc.dram_tensor("out", [NT, D], f32, kind="ExternalOutput")

    kvwR = kvwT.rearrange("(c p) d -> p c d", p=P)

    with TileContext(nc) as tc, ExitStack() as ctx:
        consts = ctx.enter_context(tc.tile_pool(name="consts", bufs=1))
        persist = ctx.enter_context(tc.tile_pool(name="persist", bufs=1))
        wtp = ctx.enter_context(tc.tile_pool(name="wtp", bufs=1))
        xfp = ctx.enter_context(tc.tile_pool(name="xfp", bufs=1))
        xfb = ctx.enter_context(tc.tile_pool(name="xfb", bufs=3))
        qfb = ctx.enter_context(tc.tile_pool(name="qfb", bufs=3))
        expp = ctx.enter_context(tc.tile_pool(name="expp", bufs=2))
        ofp = ctx.enter_context(tc.tile_pool(name="ofp", bufs=2))
        otp = ctx.enter_context(tc.tile_pool(name="otp", bufs=2))
        nrm = ctx.enter_context(tc.tile_pool(name="nrm", bufs=2))
        ps2 = ctx.enter_context(tc.tile_pool(name="ps2", bufs=3, space="PSUM"))
        pvps = ctx.enter_context(tc.tile_pool(name="pvps", bufs=2, space="PSUM"))

        ones_bf = consts.tile([1, P], bf16, tag="ones_bf")
        nc.gpsimd.memset(ones_bf[:], 1.0)
        qb_sb = kb_sb = vb_sb = pb_sb = None
        if qb_nz:
            qb_sb = consts.tile([P, DC], f32, tag="qb")
            nc.sync.dma_start(qb_sb[:], qb[:])
        if kb_nz:
            kb_sb = consts.tile([P, DC], f32, tag="kb")
            nc.sync.dma_start(kb_sb[:], kb[:])
        if vb_nz:
            vb_sb = consts.tile([1, D], bf16, tag="vb")
            nc.sync.dma_start(vb_sb[:], vb[:])
        if pb_nz:
            pb_sb = consts.tile([1, D], bf16, tag="pb")
            nc.sync.dma_start(pb_sb[:], pb[:])

        # persistent tensors
        k_feat = [persist.tile([P, NT], bf16, tag=f"k_feat{c}", name=f"k_feat{c}")
                  for c in range(DC)]
        v_pair = [persist.tile([P, 2, H, P], e4, tag=f"v{i}", name=f"v{i}")
                  for i in range(NPAIR)]
        v_rem = persist.tile([REMW, H, P], e4, tag="vrem", name="v_rem")
        for i in range(NPAIR):
            nc.gpsimd.memset(v_pair[i][:, :, :, HD:], 0.0)
            nc.gpsimd.memset(v_pair[i][:, :, :, HD:HD + 1], 1.0)
        nc.gpsimd.memset(v_rem[:, :, HD:], 0.0)
        nc.gpsimd.memset(v_rem[:, :, HD:HD + 1], 1.0)

        kvw_sb = wtp.tile([P, DC, 2 * D], bf16, tag="kvw", name="kvw")
        qw_sb = wtp.tile([P, DC, D], bf16, tag="qw", name="qw")
        pw_sb = wtp.tile([P, DC, D], bf16, tag="pw", name="pw")
        xs_feat = xfp.tile([P, DC, NT], bf16, tag="xsf", name="xsf")

        # ---- DMA emission order = execution order on the queue ----
        # xs block 0 slices + kvw K-half m-chunks first so K-proj starts
        # ~3us in; everything else follows in need-order.
        xsR = xsT.rearrange("(c p) n -> p c n", p=P)
        (n0_0, nw_0) = NQ_BLOCKS[0]
        nc.sync.dma_start(kvw_sb[:, :, :D // 2], kvwR[:, :, :D // 2])
        nc.sync.dma_start(xs_feat[:, :, n0_0:n0_0 + nw_0],
                          xsR[:, :, n0_0:n0_0 + nw_0])
        nc.sync.dma_start(kvw_sb[:, :, D // 2:D], kvwR[:, :, D // 2:D])
        nc.sync.dma_start(xs_feat[:, :, nw_0:], xsR[:, :, nw_0:])
        nc.sync.dma_start(qw_sb[:], qwT.rearrange("(c p) d -> p c d", p=P))
        nc.sync.dma_start(kvw_sb[:, :, D:], kvwR[:, :, D:])
        nc.sync.dma_start(pw_sb[:], projT.rearrange("(c p) d -> p c d", p=P))

        xa_feat = {}   # b -> list of tiles
        q_feat = {}    # b -> list of tiles

        xaR = xaT.rearrange("(c p) n -> p c n", p=P)

        def emit_xa_dma(b):
            n0, nw = NQ_BLOCKS[b]
            t = xfb.tile([P, DC, NQB], bf16, tag="xaf", name="xaf")
            nc.sync.dma_start(t[:, :, :nw], xaR[:, :, n0:n0 + nw])
            xa_feat[b] = t

        # ---------------- K projection (feature-major bf16) ----------------
        for (n0, nw) in NQ_BLOCKS:
            for mp in range(DC // 2):
                ps = ps2.tile([P, 2, NQB], f32, tag="pair", name="kproj")
                for j in range(2):
                    m = 2 * mp + j
                    for c in range(DC):
                        nc.tensor.matmul(ps[:, j, :nw],
                                         kvw_sb[:, c, m * P:(m + 1) * P],
                                         xs_feat[:, c, n0:n0 + nw],
                                         start=(c == 0), stop=(c == DC - 1))
                for j in range(2):
                    m = 2 * mp + j
                    dst = k_feat[m][:, n0:n0 + nw]
                    if kb_nz:
                        nc.vector.tensor_scalar(
                            dst, ps[:, j, :nw], kb_sb[:, m:m + 1], None, Alu.add)
                    else:
                        nc.vector.tensor_copy(dst, ps[:, j, :nw])

        # ---------------- Q projection for one block ----------------
        def emit_qproj(b, mp):
            n0, nw = NQ_BLOCKS[b]
            if mp == 0:
                q_feat[b] = [qfb.tile([P, NQB], bf16, tag=f"qf{c}", name=f"qf{c}")
                             for c in range(DC)]
            ps = ps2.tile([P, 2, NQB], f32, tag="pair", name="qproj")
            for j in range(2):
                m = 2 * mp + j
                for c in range(DC):
                    nc.tensor.matmul(ps[:, j, :nw],
                                     qw_sb[:, c, m * P:(m + 1) * P],
                                     xa_feat[b][:, c, :nw],
                                     start=(c == 0), stop=(c == DC - 1))
            for j in range(2):
                m = 2 * mp + j
                if qb_nz:
                    nc.scalar.activation(q_feat[b][m][:, :nw], ps[:, j, :nw],
                                         AF.Identity, bias=qb_sb[:, m:m + 1],
                                         scale=A8)
                else:
                    nc.scalar.activation(q_feat[b][m][:, :nw], ps[:, j, :nw],
                                         AF.Copy, scale=A8)

        # ---------------- V projection -> fp8 interleaved ----------------
        for ti, (t0, tw) in enumerate(TOK_CHUNKS):
            ps = ps2.tile([P, 2, NQB], f32, tag="pair", name="vproj")
            for half in range(2):
                for c in range(DC):
                    nc.tensor.matmul(
                        ps[:tw, half, :384],
                        xs_feat[:, c, t0:t0 + tw],
                        kvw_sb[:, c, D + half * 384:D + (half + 1) * 384],
                        start=(c == 0), stop=(c == DC - 1 and not vb_nz))
                if vb_nz:
                    nc.tensor.matmul(
                        ps[:tw, half, :384], ones_bf[:, :tw],
                        vb_sb[:, half * 384:(half + 1) * 384],
                        start=False, stop=True)
                if ti < 2 * NPAIR:
                    dst = v_pair[ti // 2][:tw, ti % 2,
                                          half * 6:(half + 1) * 6, :HD]
                else:
                    dst = v_rem[:tw, half * 6:(half + 1) * 6, :HD]
                nc.scalar.activation(
                    dst, ps[:tw, half, :384].rearrange("p (h d) -> p h d", d=HD),
                    AF.Copy)

        out_feat = {}  # b -> list of tiles
        ostate = {}    # (b, chunk) -> (ps, ot)

        def emit_ohalf(b, chunk, half):
            n0, nw = NQ_BLOCKS[b]
            c0 = chunk * P
            cw = min(P, nw - c0)
            if cw <= 0:
                return
            if half == 0:
                ps = ps2.tile([P, 2, NQB], f32, tag="pair", name="oproj")
                ot = otp.tile([P, D], f32, tag="ot", name="ot")
                ostate[(b, chunk)] = (ps, ot)
            ps, ot = ostate[(b, chunk)]
            for c in range(DC):
                nc.tensor.matmul(
                    ps[:cw, half, :384],
                    out_feat[b][c][:, c0:c0 + cw],
                    pw_sb[:, c, half * 384:(half + 1) * 384],
                    start=(c == 0), stop=(c == DC - 1 and not pb_nz))
            if pb_nz:
                nc.tensor.matmul(
                    ps[:cw, half, :384], ones_bf[:, :cw],
                    pb_sb[:, half * 384:(half + 1) * 384],
                    start=False, stop=True)
            if half == 0:
                nc.scalar.activation(ot[:cw, :384], ps[:cw, 0, :384], AF.Copy)
            else:
                nc.vector.tensor_copy(ot[:cw, 384:], ps[:cw, 1, :384])
                nc.sync.dma_start(out[n0 + c0:n0 + c0 + cw, :], ot[:cw, :])
                del ostate[(b, chunk)]

        # ---------------- flat block pipeline ----------------
        emit_xa_dma(0)
        for mp in range(DC // 2):
            emit_qproj(0, mp)

        NB = len(NQ_BLOCKS)
        for b in range(NB):
            n0, nw = NQ_BLOCKS[b]
            if b == 0:
                emit_xa_dma(1)
            if b + 2 < NB:
                emit_xa_dma(b + 2)
            out_feat[b] = [ofp.tile([P, NQB], bf16, tag=f"of{c}", name=f"of{c}")
                           for c in range(DC)]

            # PE filler slots: O-proj halves of block b-1 + Q-proj m-pairs
            # of block b+1, spread across the head pipeline.
            slots = []
            if b > 0:
                pn0, pnw = NQ_BLOCKS[b - 1]
                nchunks = (pnw + P - 1) // P
                for chunk in range(nchunks):
                    slots.append(("o", b - 1, chunk, 0))
                    slots.append(("o", b - 1, chunk, 1))
            if b == 0:
                for bb in (1, 2):
                    if bb < NB:
                        for mp in range(DC // 2):
                            slots.append(("q", bb, mp, None))
            elif b + 2 < NB:
                for mp in range(DC // 2):
                    slots.append(("q", b + 2, mp, None))


            exps = {}
            pvs = {}
            bcs = {}
            si = 0
            for it in range(H + 2):
                if it >= 2:
                    h = it - 2
                    hc, hp = h // 2, (h % 2) * HD
                    nc.vector.scalar_tensor_tensor(
                        out_feat[b][hc][hp:hp + HD, :nw],
                        pvs.pop(h)[:HD, :nw], 1.0, bcs.pop(h)[:, :nw],
                        Alu.mult, Alu.mult)

                def emit_score_pair(h, i):
                    hc, hp = h // 2, (h % 2) * HD
                    ps = ps2.tile([P, 2, NQB], f32, tag="pair", name="score")
                    for j in range(2):
                        t0 = 256 * i + 128 * j
                        nc.tensor.matmul(
                            ps[:, j, :nw],
                            k_feat[hc][hp:hp + HD, t0:t0 + P],
                            q_feat[b][hc][hp:hp + HD, :nw],
                            start=True, stop=True)
                    et = expp.tile([P, 2, NQB], e5, tag=f"e{i}", name="exp")
                    if i in (0, 3):
                        nc.vector.tensor_scalar(
                            et.bitcast(u8)[:, :, :nw], ps[:, :, :nw],
                            B8, B8CLIP, Alu.add, Alu.min)
                    else:
                        nc.scalar.activation(et[:, :, :nw], ps[:, :, :nw],
                                             AF.Exp, scale=LN2_8)
                    return et

                def emit_score_rem(h):
                    hc, hp = h // 2, (h % 2) * HD
                    psr = pvps.tile([P, NQB], f32, tag="pv", name="screm")
                    nc.tensor.matmul(psr[:REMW, :nw],
                                     k_feat[hc][hp:hp + HD, REM0:NT],
                                     q_feat[b][hc][hp:hp + HD, :nw],
                                     start=True, stop=True)
                    etr = expp.tile([REMW, NQB], e5, tag="er", name="expr")
                    if h % 2 == 0:
                        nc.vector.tensor_scalar(
                            etr.bitcast(u8)[:, :nw], psr[:REMW, :nw],
                            B8, B8CLIP, Alu.add, Alu.min)
                    else:
                        nc.scalar.activation(etr[:, :nw], psr[:REMW, :nw],
                                             AF.Exp, scale=LN2_8)
                    return etr

                # scores for head `it`
                if it < H:
                    h = it
                    exp_t = [emit_score_pair(h, i) for i in range(NPAIR)]
                    exps[h] = (exp_t, emit_score_rem(h))

                # one PE filler slot per iteration
                if si < len(slots):
                    kind, bb, xx, yy = slots[si]
                    si += 1
                    if kind == "o":
                        emit_ohalf(bb, xx, yy)
                    else:
                        emit_qproj(bb, xx)

                # PV + normalization chain for head it-1
                if 1 <= it <= H:
                    hh = it - 1
                    pexp_t, petr = exps.pop(hh)
                    pv = pvps.tile([P, NQB], f32, tag="pv", name="pv")
                    for i in range(NPAIR):
                        nc.tensor.matmul(pv[:, :nw],
                                         v_pair[i][:, :, hh, :],
                                         pexp_t[i][:, :, :nw],
                                         start=(i == 0), stop=False,
                                         perf_mode=DR)
                    nc.tensor.matmul(pv[:, :nw], v_rem[:, hh, :],
                                     petr[:, :nw], start=False, stop=True)
                    pvs[hh] = pv
                    rec = nrm.tile([1, NQB], f32, tag="rec", name="rec")
                    nc.vector.reciprocal(rec[:, :nw], pv[HD:HD + 1, :nw])
                    bc = nrm.tile([HD, NQB], f32, tag="bc", name="bc", bufs=3)
                    nc.gpsimd.partition_broadcast(bc[:, :nw], rec[:, :nw])
                    bcs[hh] = bc

            while si < len(slots):
                kind, bb, xx, yy = slots[si]
                si += 1
                if kind == "o":
                    emit_ohalf(bb, xx, yy)
                else:
                    emit_qproj(bb, xx)

        # epilogue: O-projection of the final block
        bl = NB - 1
        n0, nw = NQ_BLOCKS[bl]
        for chunk in range((nw + P - 1) // P):
            emit_ohalf(bl, chunk, 0)
            emit_ohalf(bl, chunk, 1)

    nc.finalize()
    return nc


def kernel(**inputs) -> np.ndarray:
    import ml_dtypes
    bf = ml_dtypes.bfloat16

    s_x = np.asarray(inputs["s_x"], np.float32)
    audio = np.asarray(inputs["audio"], np.float32)
    q_w = np.asarray(inputs["q_w"], np.float32)
    q_b = np.asarray(inputs["q_b"], np.float32)
    kv_w = np.asarray(inputs["kv_w"], np.float32)
    kv_b = np.asarray(inputs["kv_b"], np.float32)
    proj_w = np.asarray(inputs["proj_w"], np.float32)
    proj_b = np.asarray(inputs["proj_b"], np.float32)

    # host prep: layout + O(N*D) positional add + bf16 casts only
    pos_s = (np.asarray(inputs["clip_space_pos"], np.float32)[:, None, :]
             + np.asarray(inputs["clip_temporal_pos"], np.float32)[None, :, :]
             ).reshape(NT, D)
    pos_a = (np.asarray(inputs["audio_space_pos"], np.float32)[:, None, :]
             + np.asarray(inputs["audio_temporal_pos"], np.float32)[None, :, :]
             ).reshape(NT, D)
    qwT = np.ascontiguousarray(q_w.T).astype(bf)
    kvwT = np.ascontiguousarray(kv_w.T).astype(bf)
    projT = np.ascontiguousarray(proj_w.T).astype(bf)
    qb_nz = bool(np.any(q_b))
    kb_nz = bool(np.any(kv_b[:D]))
    vb_nz = bool(np.any(kv_b[D:]))
    pb_nz = bool(np.any(proj_b))

    key = (qb_nz, kb_nz, vb_nz, pb_nz)
    if key not in _CACHE:
        _CACHE[key] = _build_nc(*key)
    nc = _CACHE[key]

    shared = {"qwT": qwT, "kvwT": kvwT, "projT": projT}
    if qb_nz:
        shared["qb"] = np.ascontiguousarray(
            (q_b * A8).reshape(DC, P).T.astype(np.float32))
    if kb_nz:
        shared["kb"] = np.ascontiguousarray(kv_b[:D].reshape(DC, P).T)
    if vb_nz:
        shared["vb"] = np.ascontiguousarray(kv_b[D:].reshape(1, D)).astype(bf)
    if pb_nz:
        shared["pb"] = np.ascontiguousarray(proj_b.reshape(1, D)).astype(bf)

    in_maps = []
    for b in range(N_CORES):
        m = dict(shared)
        m["xsT"] = np.ascontiguousarray(
            (s_x[1:, b * T:(b + 1) * T, :].reshape(NT, D) + pos_s).T).astype(bf)
        m["xaT"] = np.ascontiguousarray(
            (audio[2:, b * T:(b + 1) * T, :].reshape(NT, D) + pos_a).T).astype(bf)
        in_maps.append(m)

    from concourse.bass_utils import run_bass_kernel_spmd
    res = run_bass_kernel_spmd(nc, in_maps, core_ids=list(range(N_CORES)))
    global LAST_RESULTS
    LAST_RESULTS = res

    out_full = np.empty((2 + APATCH, B * T, D), np.float32)
    out_full[:2] = audio[:2]
    for b in range(N_CORES):
        out_full[2:, b * T:(b + 1) * T, :] = \
            res.results[b]["out"].reshape(APATCH, T, D)
    return out_full


# revision 53
# speedup vs baseline: 1.0099x; 1.0033x over previous
"""Cross-attention (S2Audio) Trainium2 Bass kernel.

Sharding: data-parallel over the clip batch B=8 -> one batch element per
NeuronCore.  Per core, for its batch element b:

  q = (audio_patch + pos_a) @ q_w.T + q_b          (1568, 768)
  k,v = (s_x_patch + pos_s) @ kv_w.T + kv_b        (1568, 768) each
  out = softmax(q k^T / sqrt(64)) v  per 12 heads  -> proj -> (1568, 768)

Engine strategy (per core):
  * All projections + scores matmuls in bf16 (1 cycle/row on PE), fp32 PSUM.
  * PV matmul in fp8 with DoubleRow perf mode (0.5 cycles/row): V in e4m3,
    interleaved [128, 2, H, 128] per 256-key pair (v dims 0-63, ones at
    col 64 -> softmax denominator in pv row 64, zero padding above — DR
    weight tiles must be exactly 32/64/128 columns).  exp tiles are e5m2
    in the DoubleRow rhs layout [128, 2, nq]: e5m2's 2^31 dynamic range
    covers exp of the full score range (+-9.5) with no max-subtraction
    and no overflow (e4m3 exp overflowed to NaN bits / flushed whole
    rows to zero).
  * q_feat is pre-scaled by A8 = SCALE*4*log2(e) so the scores PSUM holds
    s*A8.  exp splits across the two PSUM-capable engines (GpSimd cannot
    read PSUM):
      - ACT:  exact exp via activation(Exp, scale=ln2/4) -> e5m2 out
      - DVE:  Schraudolph bit trick: min(round(psum + B8), 123) as
              saturating-uint8 through a bitcast view = e5m2 bits of exp
  * Softmax normalization: DVE reciprocal of the denominator row, GpSimd
    partition_broadcast (SBUF only), DVE scalar_tensor_tensor multiply.
  * One flat pipeline, no phase drains: scores/PV software-pipelined by
    one head; Q-proj of the next block and O-proj of the previous block
    are interleaved into the head loop as PE filler so the PE never
    outruns the exp engines (PSUM pair rotation stays unblocked) and the
    PE p-state stays at full clock.  DMAs are consolidated (HWDGE costs
    ~625ns per dma_start regardless of size).
  * PSUM budget: score-pair tiles [128, 2, 512] x3 (6 banks) + pv x2 = 8.
  * TimelineSim: 292.3us/core (baseline 458.1us); measured rel err 9.9e-3.
"""

import numpy as np
from contextlib import ExitStack

B, T, NPATCH, APATCH, D, H = 8, 8, 196, 196, 768, 12
HD = D // H                      # 64
SCALE = float(HD) ** -0.5        # 0.125
NT = NPATCH * T                  # 1568 tokens (both q and kv side)
P = 128
DC = D // P                      # 6 feature chunks
N_CORES = 8

NPAIR = 6                        # 6 x 256-key DoubleRow pairs
REM0 = NPAIR * 256               # 1536
REMW = NT - REM0                 # 32 remainder keys
NQB = 512
NQ_BLOCKS = [(s, min(NQB, NT - s)) for s in range(0, NT, NQB)]
TOK_CHUNKS = [(i * P, min(P, NT - i * P)) for i in range((NT + P - 1) // P)]

A8 = SCALE * 4.0 * float(np.log2(np.e))   # q prescale so psum = s_raw*A8
LN2_8 = float(np.log(2.0)) / 4.0          # ACT exp scale on prescaled psum
B8 = 60.0 - 0.5                           # Schraudolph e5m2 offset (tuned)
B8CLIP = 123.0                            # e5m2 bits >= 124 are inf/nan

_CACHE: dict = {}


def _build_nc(qb_nz: bool, kb_nz: bool, vb_nz: bool, pb_nz: bool):
    import concourse.mybir as mybir
    from concourse import bacc
    from concourse.tile import TileContext

    f32 = mybir.dt.float32
    bf16 = mybir.dt.bfloat16
    e4 = mybir.dt.float8e4
    e5 = mybir.dt.float8e5
    u8 = mybir.dt.uint8
    AF = mybir.ActivationFunctionType
    Alu = mybir.AluOpType
    DR = mybir.MatmulPerfMode.DoubleRow

    nc = bacc.Bacc("TRN2", target_bir_lowering=False, debug=False,
                   num_devices=N_CORES)

    xsT = nc.dram_tensor("xsT", [D, NT], bf16, kind="ExternalInput")
    xaT = nc.dram_tensor("xaT", [D, NT], bf16, kind="ExternalInput")
    qwT = nc.dram_tensor("qwT", [D, D], bf16, kind="ExternalInput")
    kvwT = nc.dram_tensor("kvwT", [D, 2 * D], bf16, kind="ExternalInput")
    projT = nc.dram_tensor("projT", [D, D], bf16, kind="ExternalInput")
    qb = nc.dram_tensor("qb", [P, DC], f32, kind="ExternalInput") if qb_nz else None
    kb = nc.dram_tensor("kb", [P, DC], f32, kind="ExternalInput") if kb_nz else None
    vb = nc.dram_tensor("vb", [1, D], bf16, kind="ExternalInput") if vb_nz else None
    pb = nc.dram_tensor("pb", [1, D], bf16, kind="ExternalInput") if pb_nz else None
    out = nc.dram_tensor("out", [NT, D], f32, kind="ExternalOutput")

    kvwR = kvwT.rearrange("(c p) d -> p c d", p=P)

    with TileContext(nc) as tc, ExitStack() as ctx:
        consts = ctx.enter_context(tc.tile_pool(name="consts", bufs=1))
        persist = ctx.enter_context(tc.tile_pool(name="persist", bufs=1))
        wtp = ctx.enter_context(tc.tile_pool(name="wtp", bufs=1))
        xfp = ctx.enter_context(tc.tile_pool(name="xfp", bufs=1))
        xfb = ctx.enter_context(tc.tile_pool(name="xfb", bufs=3))
        qfb = ctx.enter_context(tc.tile_pool(name="qfb", bufs=3))
        expp = ctx.enter_context(tc.tile_pool(name="expp", bufs=2))
        ofp = ctx.enter_context(tc.tile_pool(name="ofp", bufs=2))
        otp = ctx.enter_context(tc.tile_pool(name="otp", bufs=2))
        nrm = ctx.enter_context(tc.tile_pool(name="nrm", bufs=2))
        ps2 = ctx.enter_context(tc.tile_pool(name="ps2", bufs=3, space="PSUM"))
        pvps = ctx.enter_context(tc.tile_pool(name="pvps", bufs=2, space="PSUM"))

        ones_bf = consts.tile([1, P], bf16, tag="ones_bf")
        nc.gpsimd.memset(ones_bf[:], 1.0)
        qb_sb = kb_sb = vb_sb = pb_sb = None
        if qb_nz:
            qb_sb = consts.tile([P, DC], f32, tag="qb")
            nc.sync.dma_start(qb_sb[:], qb[:])
        if kb_nz:
            kb_sb = consts.tile([P, DC], f32, tag="kb")
            nc.sync.dma_start(kb_sb[:], kb[:])
        if vb_nz:
            vb_sb = consts.tile([1, D], bf16, tag="vb")
            nc.sync.dma_start(vb_sb[:], vb[:])
        if pb_nz:
            pb_sb = consts.tile([1, D], bf16, tag="pb")
            nc.sync.dma_start(pb_sb[:], pb[:])

        # persistent tensors
        k_feat = [persist.tile([P, NT], bf16, tag=f"k_feat{c}", name=f"k_feat{c}")
                  for c in range(DC)]
        v_pair = [persist.tile([P, 2, H, P], e4, tag=f"v{i}", name=f"v{i}")
                  for i in range(NPAIR)]
        v_rem = persist.tile([REMW, H, P], e4, tag="vrem", name="v_rem")
        for i in range(NPAIR):
            nc.gpsimd.memset(v_pair[i][:, :, :, HD:], 0.0)
            nc.gpsimd.memset(v_pair[i][:, :, :, HD:HD + 1], 1.0)
        nc.gpsimd.memset(v_rem[:, :, HD:], 0.0)
        nc.gpsimd.memset(v_rem[:, :, HD:HD + 1], 1.0)

        kvw_sb = wtp.tile([P, DC, 2 * D], bf16, tag="kvw", name="kvw")
        qw_sb = wtp.tile([P, DC, D], bf16, tag="qw", name="qw")
        pw_sb = wtp.tile([P, DC, D], bf16, tag="pw", name="pw")
        xs_feat = xfp.tile([P, DC, NT], bf16, tag="xsf", name="xsf")

        # ---- DMA emission order = execution order on the queue ----
        # xs block 0 slices + kvw K-half m-chunks first so K-proj starts
        # ~3us in; everything else follows in need-order.
        xsR = xsT.rearrange("(c p) n -> p c n", p=P)
        (n0_0, nw_0) = NQ_BLOCKS[0]
        nc.sync.dma_start(kvw_sb[:, :, :D // 2], kvwR[:, :, :D // 2])
        nc.sync.dma_start(xs_feat[:, :3, n0_0:n0_0 + nw_0],
                          xsR[:, :3, n0_0:n0_0 + nw_0])
        nc.sync.dma_start(xs_feat[:, 3:, n0_0:n0_0 + nw_0],
                          xsR[:, 3:, n0_0:n0_0 + nw_0])
        nc.sync.dma_start(kvw_sb[:, :, D // 2:D], kvwR[:, :, D // 2:D])
        nc.sync.dma_start(xs_feat[:, :, nw_0:], xsR[:, :, nw_0:])
        nc.sync.dma_start(qw_sb[:], qwT.rearrange("(c p) d -> p c d", p=P))
        nc.sync.dma_start(kvw_sb[:, :, D:], kvwR[:, :, D:])
        nc.sync.dma_start(pw_sb[:], projT.rearrange("(c p) d -> p c d", p=P))

        xa_feat = {}   # b -> list of tiles
        q_feat = {}    # b -> list of tiles

        xaR = xaT.rearrange("(c p) n -> p c n", p=P)

        def emit_xa_dma(b):
            n0, nw = NQ_BLOCKS[b]
            t = xfb.tile([P, DC, NQB], bf16, tag="xaf", name="xaf")
            nc.sync.dma_start(t[:, :, :nw], xaR[:, :, n0:n0 + nw])
            xa_feat[b] = t

        # ---------------- K projection (feature-major bf16) ----------------
        for (n0, nw) in NQ_BLOCKS:
            for mp in range(DC // 2):
                ps = ps2.tile([P, 2, NQB], f32, tag="pair", name="kproj")
                for j in range(2):
                    m = 2 * mp + j
                    for c in range(DC):
                        nc.tensor.matmul(ps[:, j, :nw],
                                         kvw_sb[:, c, m * P:(m + 1) * P],
                                         xs_feat[:, c, n0:n0 + nw],
                                         start=(c == 0), stop=(c == DC - 1))
                for j in range(2):
                    m = 2 * mp + j
                    dst = k_feat[m][:, n0:n0 + nw]
                    if kb_nz:
                        nc.vector.tensor_scalar(
                            dst, ps[:, j, :nw], kb_sb[:, m:m + 1], None, Alu.add)
                    else:
                        nc.vector.tensor_copy(dst, ps[:, j, :nw])

        # ---------------- Q projection for one block ----------------
        def emit_qproj(b, mp):
            n0, nw = NQ_BLOCKS[b]
            if mp == 0:
                q_feat[b] = [qfb.tile([P, NQB], bf16, tag=f"qf{c}", name=f"qf{c}")
                             for c in range(DC)]
            ps = ps2.tile([P, 2, NQB], f32, tag="pair", name="qproj")
            for j in range(2):
                m = 2 * mp + j
                for c in range(DC):
                    nc.tensor.matmul(ps[:, j, :nw],
                                     qw_sb[:, c, m * P:(m + 1) * P],
                                     xa_feat[b][:, c, :nw],
                                     start=(c == 0), stop=(c == DC - 1))
            for j in range(2):
                m = 2 * mp + j
                if qb_nz:
                    nc.scalar.activation(q_feat[b][m][:, :nw], ps[:, j, :nw],
                                         AF.Identity, bias=qb_sb[:, m:m + 1],
                                         scale=A8)
                else:
                    nc.scalar.activation(q_feat[b][m][:, :nw], ps[:, j, :nw],
                                         AF.Copy, scale=A8)

        # ---------------- V projection -> fp8 interleaved ----------------
        for ti, (t0, tw) in enumerate(TOK_CHUNKS):
            ps = ps2.tile([P, 2, NQB], f32, tag="pair", name="vproj")
            for half in range(2):
                for c in range(DC):
                    nc.tensor.matmul(
                        ps[:tw, half, :384],
                        xs_feat[:, c, t0:t0 + tw],
                        kvw_sb[:, c, D + half * 384:D + (half + 1) * 384],
                        start=(c == 0), stop=(c == DC - 1 and not vb_nz))
                if vb_nz:
                    nc.tensor.matmul(
                        ps[:tw, half, :384], ones_bf[:, :tw],
                        vb_sb[:, half * 384:(half + 1) * 384],
                        start=False, stop=True)
                if ti < 2 * NPAIR:
                    dst = v_pair[ti // 2][:tw, ti % 2,
                                          half * 6:(half + 1) * 6, :HD]
                else:
                    dst = v_rem[:tw, half * 6:(half + 1) * 6, :HD]
                nc.scalar.activation(
                    dst, ps[:tw, half, :384].rearrange("p (h d) -> p h d", d=HD),
                    AF.Copy)

        out_feat = {}  # b -> list of tiles
        ostate = {}    # (b, chunk) -> (ps, ot)

        def emit_ohalf(b, chunk, half):
            n0, nw = NQ_BLOCKS[b]
            c0 = chunk * P
            cw = min(P, nw - c0)
            if cw <= 0:
                return
            if half == 0:
                ps = ps2.tile([P, 2, NQB], f32, tag="pair", name="oproj")
                ot = otp.tile([P, D], f32, tag="ot", name="ot")
                ostate[(b, chunk)] = (ps, ot)
            ps, ot = ostate[(b, chunk)]
            for c in range(DC):
                nc.tensor.matmul(
                    ps[:cw, half, :384],
                    out_feat[b][c][:, c0:c0 + cw],
                    pw_sb[:, c, half * 384:(half + 1) * 384],
                    start=(c == 0), stop=(c == DC - 1 and not pb_nz))
            if pb_nz:
                nc.tensor.matmul(
                    ps[:cw, half, :384], ones_bf[:, :cw],
                    pb_sb[:, half * 384:(half + 1) * 384],
                    start=False, stop=True)
            if half == 0:
                nc.scalar.activation(ot[:cw, :384], ps[:cw, 0, :384], AF.Copy)
            else:
                nc.vector.tensor_copy(ot[:cw, 384:], ps[:cw, 1, :384])
                nc.sync.dma_start(out[n0 + c0:n0 + c0 + cw, :], ot[:cw, :])
                del ostate[(b, chunk)]

        # ---------------- flat block pipeline ----------------
        emit_xa_dma(0)
        for mp in range(DC // 2):
            emit_qproj(0, mp)

        NB = len(NQ_BLOCKS)
        for b in range(NB):
            n0, nw = NQ_BLOCKS[b]
            if b == 0:
                emit_xa_dma(1)
            if b + 2 < NB:
                emit_xa_dma(b + 2)
            out_feat[b] = [ofp.tile([P, NQB], bf16, tag=f"of{c}", name=f"of{c}")
                           for c in range(DC)]

            # PE filler slots: O-proj halves of block b-1 + Q-proj m-pairs
            # of block b+1, spread across the head pipeline.
            slots = []
            if b > 0:
                pn0, pnw = NQ_BLOCKS[b - 1]
                nchunks = (pnw + P - 1) // P
                for chunk in range(nchunks):
                    slots.append(("o", b - 1, chunk, 0))
                    slots.append(("o", b - 1, chunk, 1))
            if b == 0:
                for bb in (1, 2):
                    if bb < NB:
                        for mp in range(DC // 2):
                            slots.append(("q", bb, mp, None))
            elif b + 2 < NB:
                for mp in range(DC // 2):
                    slots.append(("q", b + 2, mp, None))


            exps = {}
            pvs = {}
            bcs = {}
            si = 0
            for it in range(H + 2):
                if it >= 2:
                    h = it - 2
                    hc, hp = h // 2, (h % 2) * HD
                    nc.vector.scalar_tensor_tensor(
                        out_feat[b][hc][hp:hp + HD, :nw],
                        pvs.pop(h)[:HD, :nw], 1.0, bcs.pop(h)[:, :nw],
                        Alu.mult, Alu.mult)

                def emit_score_pair(h, i):
                    hc, hp = h // 2, (h % 2) * HD
                    ps = ps2.tile([P, 2, NQB], f32, tag="pair", name="score")
                    for j in range(2):
                        t0 = 256 * i + 128 * j
                        nc.tensor.matmul(
                            ps[:, j, :nw],
                            k_feat[hc][hp:hp + HD, t0:t0 + P],
                            q_feat[b][hc][hp:hp + HD, :nw],
                            start=True, stop=True)
                    et = expp.tile([P, 2, NQB], e5, tag=f"e{i}", name="exp")
                    if i in (0, 3):
                        nc.vector.tensor_scalar(
                            et.bitcast(u8)[:, :, :nw], ps[:, :, :nw],
                            B8, B8CLIP, Alu.add, Alu.min)
                    else:
                        nc.scalar.activation(et[:, :, :nw], ps[:, :, :nw],
                                             AF.Exp, scale=LN2_8)
                    return et

                def emit_score_rem(h):
                    hc, hp = h // 2, (h % 2) * HD
                    psr = pvps.tile([P, NQB], f32, tag="pv", name="screm")
                    nc.tensor.matmul(psr[:REMW, :nw],
                                     k_feat[hc][hp:hp + HD, REM0:NT],
                                     q_feat[b][hc][hp:hp + HD, :nw],
                                     start=True, stop=True)
                    etr = expp.tile([REMW, NQB], e5, tag="er", name="expr")
                    if h % 2 == 0:
                        nc.vector.tensor_scalar(
                            etr.bitcast(u8)[:, :nw], psr[:REMW, :nw],
                            B8, B8CLIP, Alu.add, Alu.min)
                    else:
                        nc.scalar.activation(etr[:, :nw], psr[:REMW, :nw],
                                             AF.Exp, scale=LN2_8)
                    return etr

                # scores for head `it`
                if it < H:
                    h = it
                    exp_t = [emit_score_pair(h, i) for i in range(NPAIR)]
                    exps[h] = (exp_t, emit_score_rem(h))

                # one PE filler slot per iteration
                if si < len(slots):
                    kind, bb, xx, yy = slots[si]
                    si += 1
                    if kind == "o":
                        emit_ohalf(bb, xx, yy)
                    else:
                        emit_qproj(bb, xx)

                # PV + normalization chain for head it-1
                if 1 <= it <= H:
                    hh = it - 1
                    pexp_t, petr = exps.pop(hh)
                    pv = pvps.tile([P, NQB], f32, tag="pv", name="pv")
                    for i in range(NPAIR):
                        nc.tensor.matmul(pv[:, :nw],
                                         v_pair[i][:, :, hh, :],
                                         pexp_t[i][:, :, :nw],
                                         start=(i == 0), stop=False,
                                         perf_mode=DR)
                    nc.tensor.matmul(pv[:, :nw], v_rem[:, hh, :],
                                     petr[:, :nw], start=False, stop=True)
                    pvs[hh] = pv
                    rec = nrm.tile([1, NQB], f32, tag="rec", name="rec")
                    nc.vector.reciprocal(rec[:, :nw], pv[HD:HD + 1, :nw])
                    bc = nrm.tile([HD, NQB], f32, tag="bc", name="bc", bufs=3)
                    nc.gpsimd.partition_broadcast(bc[:, :nw], rec[:, :nw])
                    bcs[hh] = bc

            while si < len(slots):
                kind, bb, xx, yy = slots[si]
                si += 1
                if kind == "o":
                    emit_ohalf(bb, xx, yy)
                else:
                    emit_qproj(bb, xx)

        # epilogue: O-projection of the final block
        bl = NB - 1
        n0, nw = NQ_BLOCKS[bl]
        for chunk in range((nw + P - 1) // P):
            emit_ohalf(bl, chunk, 0)
            emit_ohalf(bl, chunk, 1)

    nc.finalize()
    return nc


def kernel(**inputs) -> np.ndarray:
    import ml_dtypes
    bf = ml_dtypes.bfloat16

    s_x = np.asarray(inputs["s_x"], np.float32)
    audio = np.asarray(inputs["audio"], np.float32)
    q_w = np.asarray(inputs["q_w"], np.float32)
    q_b = np.asarray(inputs["q_b"], np.float32)
    kv_w = np.asarray(inputs["kv_w"], np.float32)
    kv_b = np.asarray(inputs["kv_b"], np.float32)
    proj_w = np.asarray(inputs["proj_w"], np.float32)
    proj_b = np.asarray(inputs["proj_b"], np.float32)

    # host prep: layout + O(N*D) positional add + bf16 casts only
    pos_s = (np.asarray(inputs["clip_space_pos"], np.float32)[:, None, :]
             + np.asarray(inputs["clip_temporal_pos"], np.float32)[None, :, :]
             ).reshape(NT, D)
    pos_a = (np.asarray(inputs["audio_space_pos"], np.float32)[:, None, :]
             + np.asarray(inputs["audio_temporal_pos"], np.float32)[None, :, :]
             ).reshape(NT, D)
    qwT = np.ascontiguousarray(q_w.T).astype(bf)
    kvwT = np.ascontiguousarray(kv_w.T).astype(bf)
    projT = np.ascontiguousarray(proj_w.T).astype(bf)
    qb_nz = bool(np.any(q_b))
    kb_nz = bool(np.any(kv_b[:D]))
    vb_nz = bool(np.any(kv_b[D:]))
    pb_nz = bool(np.any(proj_b))

    key = (qb_nz, kb_nz, vb_nz, pb_nz)
    if key not in _CACHE:
        _CACHE[key] = _build_nc(*key)
    nc = _CACHE[key]

    shared = {"qwT": qwT, "kvwT": kvwT, "projT": projT}
    if qb_nz:
        shared["qb"] = np.ascontiguousarray(
            (q_b * A8).reshape(DC, P).T.astype(np.float32))
    if kb_nz:
        shared["kb"] = np.ascontiguousarray(kv_b[:D].reshape(DC, P).T)
    if vb_nz:
        shared["vb"] = np.ascontiguousarray(kv_b[D:].reshape(1, D)).astype(bf)
    if pb_nz:
        shared["pb"] = np.ascontiguousarray(proj_b.reshape(1, D)).astype(bf)

    in_maps = []
    for b in range(N_CORES):
        m = dict(shared)
        m["xsT"] = np.ascontiguousarray(
            (s_x[1:, b * T:(b + 1) * T, :].reshape(NT, D) + pos_s).T).astype(bf)
        m["xaT"] = np.ascontiguousarray(
            (audio[2:, b * T:(b + 1) * T, :].reshape(NT, D) + pos_a).T).astype(bf)
        in_maps.append(m)

    from concourse.bass_utils import run_bass_kernel_spmd
    res = run_bass_kernel_spmd(nc, in_maps, core_ids=list(range(N_CORES)))
    global LAST_RESULTS
    LAST_RESULTS = res

    out_full = np.empty((2 + APATCH, B * T, D), np.float32)
    out_full[:2] = audio[:2]
    for b in range(N_CORES):
        out_full[2:, b * T:(b + 1) * T, :] = \
            res.results[b]["out"].reshape(APATCH, T, D)
    return out_full
